# revision 10
# baseline (speedup 1.0000x reference)
"""Trainium2 Bass kernel for nn_ContinuousEmbedding (histogram binning + distance-
weighted embedding mix).

Math: for each scalar x[b,f], the reference computes bucket index
idx = #{j in 1..63 : x > low[j]} and returns
    out[b,f,:] = sum_k weight[k,:] / (|idx-k|+1)  =  T[idx,:]
where T = S @ weight, S[i,k] = 1/(|i-k|+1) is a fixed 64x64 matrix.

T[idx] telescopes over compare signs s_j = sign(x - low[j]) (s_0 = +1 since
low[0] = -inf):
    T[idx] = sum_j s_j * V2[j],  V2[0] = (T[0]+T[63])/2, V2[j] = (T[j]-T[j-1])/2
i.e. out_row = V2^T s(x) -- a 64-deep fp16 contraction on the TensorEngine.

Per 2048-token chunk pair (32 pairs per core), tiles are [128, 1024] with chunk
A on partitions 0:64 and chunk B on 64:128:
  grid:  xb[128, 1024] = x broadcast to 64 partitions per chunk, via one of
           - gpsimd.partition_broadcast
           - DMA with a stride-0 (replicated) DRAM read
           - PE outer product from an exact 3-way fp16 split of x shipped from
             the host (hx+mx+lx == x exactly in f32 accumulation), K=3 matmul
         (mix per 8 pairs is tuned so GPSIMD / DMA / PE finish together)
  sign:  sg[128, 1024] fp16 = Sign(xb + (-low))  (ACT, per-partition bias; for
         the PE path the bias-add reads the PSUM x directly)
  gather: 4 matmuls vtab^T sg -> psum [128, 1024], diagonal PE quadrants
          (0,0)/(64,64) so both chunks' grids gather without moving data
  out:   DVE psum -> sbuf fp16 [128, 1024]; 2 HWDGE DMAs -> outT[64, NTOK]
The device output is transposed [D, tokens] fp16; the host transposes + casts
to f32 once at unshard time. Ties (x exactly equal to a bin edge) give
sign(0)=0 and are patched exactly on the host, as before.
"""

import os as _os
import sys

import numpy as np

for _p in ("/opt/trn_rl_repo",):
    if _p not in sys.path:
        sys.path.insert(0, _p)

import concourse.bass as bass  # noqa: E402,F401
import concourse.mybir as mybir  # noqa: E402
import concourse.tile as tile  # noqa: E402
from concourse import bacc  # noqa: E402
from concourse import bass_utils  # noqa: E402

B, F, K, D = 8192, 64, 64, 64
NCORES = 8
NTOK = (B // NCORES) * F          # 65536 tokens per core
CHUNK = 1024                      # tokens per chunk
NPAIR = NTOK // (2 * CHUNK)       # 32 chunk pairs
HALF = CHUNK // 2                 # tokens per matmul (N=512)

FP16 = mybir.dt.float16
F32 = mybir.dt.float32

CFG = {
    # mechanism per chunk pair, cycled: g = GPSIMD broadcast, d = DMA stride-0
    # broadcast, p = PE fp16-split outer product.
    "pat": "gdgpgdgp",
}
for _kv in _os.environ.get("KCFG", "").split(","):
    if "=" in _kv:
        _k, _v = _kv.split("=", 1)
        CFG[_k.strip()] = int(_v) if _v.strip().lstrip("-").isdigit() else _v.strip()

SIGN = mybir.ActivationFunctionType.Sign


def build_tile_kernel(nc, tc, x_d, xs_d, low_d, v_d, out_d):
    x_ap = x_d.ap().rearrange("(c n) -> c n", c=NTOK // CHUNK)       # [64, 1024]
    xs_ap = xs_d.ap().rearrange("k (p n) -> p k n", p=NPAIR)         # [32, 3, 2048]
    # fused pair store: [pair, (chunk d), n] <- ob[128, 1024]
    out_ap = out_d.ap().rearrange(
        "d (p two n) -> p two d n", two=2, n=CHUNK
    )                                                                # [32, 2, 64, 1024]

    pat = CFG["pat"]

    with tc.tile_pool(name="cpool", bufs=1) as cpool:
        neglow2 = cpool.tile([128, 1], F32)
        nc.sync.dma_start(out=neglow2[:], in_=low_d.ap())
        vtab2 = cpool.tile([128, D], FP16)
        nc.sync.dma_start(out=vtab2[:], in_=v_d.ap())
        ones3 = cpool.tile([3, K], FP16)
        nc.vector.memset(ones3[:], 1.0)

        with (
            tc.tile_pool(name="wpool", bufs=4) as wpool,
            tc.tile_pool(name="spool", bufs=4) as spool,
            tc.tile_pool(name="opool", bufs=4) as opool,
            tc.tile_pool(name="pxpool", bufs=2, space="PSUM") as pxpool,
            tc.tile_pool(name="popool", bufs=2, space="PSUM") as popool,
        ):
            for p in range(NPAIR):
                mech = pat[p % len(pat)]
                c0, c1 = 2 * p, 2 * p + 1

                if mech == "p":
                    # PE outer product from exact fp16 split; [128, 1024] packing
                    sg = spool.tile([128, CHUNK], FP16, tag="sg128")
                    xsp = wpool.tile([3, 2 * CHUNK], FP16, tag="xsp")
                    nc.sync.dma_start(out=xsp[:], in_=xs_ap[p])
                    px = pxpool.tile([128, CHUNK], F32, tag="px")
                    for h in range(2):
                        sl = slice(HALF * h, HALF * (h + 1))
                        nc.tensor.matmul(
                            out=px[0:64, sl], lhsT=ones3[:], rhs=xsp[:, sl],
                            start=True, stop=True, tile_position=(0, 0),
                        )
                        nc.tensor.matmul(
                            out=px[64:128, sl], lhsT=ones3[:],
                            rhs=xsp[:, CHUNK + HALF * h : CHUNK + HALF * (h + 1)],
                            start=True, stop=True, tile_position=(0, 64),
                        )
                    nc.scalar.activation(
                        out=sg[:], in_=px[:], func=SIGN, bias=neglow2[:], scale=1.0
                    )
                elif mech == "d":
                    # DMA stride-0 broadcast; [128, 1024] packing
                    sg = spool.tile([128, CHUNK], FP16, tag="sg128")
                    xb = wpool.tile([128, CHUNK], F32, tag="xb128")
                    nc.sync.dma_start(
                        out=xb[0:64, :],
                        in_=x_ap[c0].unsqueeze(0).broadcast_to([K, CHUNK]),
                    )
                    nc.sync.dma_start(
                        out=xb[64:128, :],
                        in_=x_ap[c1].unsqueeze(0).broadcast_to([K, CHUNK]),
                    )
                    nc.scalar.activation(
                        out=sg[:], in_=xb[:], func=SIGN, bias=neglow2[:], scale=1.0
                    )
                else:
                    # GPSIMD broadcast: must stay on partitions 0:64 -> [64, 2048]
                    sg = spool.tile([K, 2 * CHUNK], FP16, tag="sg64")
                    xb = wpool.tile([K, 2 * CHUNK], F32, tag="xb64")
                    xr0 = wpool.tile([1, CHUNK], F32, tag="xr0")
                    xr1 = wpool.tile([1, CHUNK], F32, tag="xr1")
                    nc.sync.dma_start(out=xr0[:], in_=x_ap[c0])
                    nc.sync.dma_start(out=xr1[:], in_=x_ap[c1])
                    nc.gpsimd.partition_broadcast(xb[:, 0:CHUNK], xr0[:], channels=K)
                    nc.gpsimd.partition_broadcast(
                        xb[:, CHUNK : 2 * CHUNK], xr1[:], channels=K
                    )
                    nc.scalar.activation(
                        out=sg[:], in_=xb[:], func=SIGN,
                        bias=neglow2[0:64, :], scale=1.0,
                    )

                po = popool.tile([128, CHUNK], F32, tag="po")
                for h in range(2):
                    sl = slice(HALF * h, HALF * (h + 1))
                    slB = slice(CHUNK + HALF * h, CHUNK + HALF * (h + 1))
                    if mech == "g":
                        nc.tensor.matmul(
                            out=po[0:64, sl], lhsT=vtab2[0:64, :], rhs=sg[:, sl],
                            start=True, stop=True, tile_position=(0, 0),
                        )
                        nc.tensor.matmul(
                            out=po[64:128, sl], lhsT=vtab2[0:64, :], rhs=sg[:, slB],
                            start=True, stop=True, tile_position=(0, 64),
                        )
                    else:
                        nc.tensor.matmul(
                            out=po[0:64, sl], lhsT=vtab2[0:64, :], rhs=sg[0:64, sl],
                            start=True, stop=True, tile_position=(0, 0),
                        )
                        nc.tensor.matmul(
                            out=po[64:128, sl], lhsT=vtab2[64:128, :],
                            rhs=sg[64:128, sl],
                            start=True, stop=True, tile_position=(64, 64),
                        )

                ob = opool.tile([128, CHUNK], FP16, tag="ob")
                nc.vector.tensor_copy(out=ob[:], in_=po[:])
                nc.sync.dma_start(out=out_ap[p], in_=ob[:])


_CACHED_NC = None


def _get_nc():
    global _CACHED_NC
    if _CACHED_NC is None:
        nc = bacc.Bacc("TRN2", target_bir_lowering=False, debug=False)
        x_d = nc.dram_tensor("x", [NTOK], F32, kind="ExternalInput")
        xs_d = nc.dram_tensor("xsplit", [3, NTOK], FP16, kind="ExternalInput")
        low_d = nc.dram_tensor("lowcol", [128, 1], F32, kind="ExternalInput")
        v_d = nc.dram_tensor("vtab", [128, D], FP16, kind="ExternalInput")
        out_d = nc.dram_tensor("out", [D, NTOK], FP16, kind="ExternalOutput")
        with tile.TileContext(nc) as tc:
            build_tile_kernel(nc, tc, x_d, xs_d, low_d, v_d, out_d)
        nc.compile()
        _CACHED_NC = nc
    return _CACHED_NC


def make_host_tables(low, weight):
    """V2 duplicated to [128, D] fp16 (sign-telescoped table) and -low column
    duplicated to [128, 1] f32, computed in float64."""
    ar = np.arange(K)
    S = 1.0 / (np.abs(ar[:, None] - ar[None, :]) + 1.0)              # [K, K] f64
    T = S @ np.asarray(weight, np.float64)                           # [K, D]
    V = np.empty_like(T)
    V[0] = (T[0] + T[-1]) / 2
    V[1:] = (T[1:] - T[:-1]) / 2
    vtab = V.astype(np.float16)
    vtab2 = np.concatenate([vtab, vtab], axis=0)                     # [128, D]
    lowcol = (-np.asarray(low, np.float64)).astype(np.float32).reshape(K, 1)
    lowcol2 = np.concatenate([lowcol, lowcol], axis=0)               # [128, 1]
    return lowcol2, vtab2


def split_fp16_3(xflat):
    """Exact 3-way fp16 split: hx + mx + lx == x in f32 (24 <= 3*11 mantissa
    bits; each partial sum is exactly representable)."""
    x = np.asarray(xflat, np.float32)
    hx = x.astype(np.float16)
    r = x - hx.astype(np.float32)
    mx = r.astype(np.float16)
    r2 = r - mx.astype(np.float32)
    lx = r2.astype(np.float16)
    return np.stack([hx, mx, lx], axis=0)                            # [3, NTOK]


def host_correct_ties(out2d, xflat, low, weight):
    """Exact fixup for tokens where x equals a bin edge: the device Sign gives
    sign(0)=0 there (averaging two table rows) while the reference uses strict
    x > low. Replace those few rows with the exact table row."""
    bins = np.asarray(low, np.float32)[1:]
    ties = np.isin(xflat, bins)
    if not ties.any():
        return out2d
    xt = xflat[ties]
    idx = (xt[:, None] > bins[None, :]).sum(-1)
    ar = np.arange(K)
    S = 1.0 / (np.abs(ar[:, None] - ar[None, :]) + 1.0)
    T = (S @ np.asarray(weight, np.float64)).astype(np.float32)
    out2d[ties] = T[idx]
    return out2d


def make_in_maps(x, low, weight):
    lowcol2, vtab2 = make_host_tables(low, weight)
    shards = np.asarray(x, np.float32).reshape(NCORES, NTOK)
    in_maps = []
    for i in range(NCORES):
        xi = np.ascontiguousarray(shards[i])
        in_maps.append(
            {"x": xi, "xsplit": split_fp16_3(xi), "lowcol": lowcol2, "vtab": vtab2}
        )
    return in_maps


def run_cores(x, low, weight, trace=False):
    """Shard, run on 8 cores, return ([NTOK*8, D] f32 output, BassKernelResults)."""
    nc = _get_nc()
    in_maps = make_in_maps(x, low, weight)
    res = bass_utils.run_bass_kernel_spmd(
        nc, in_maps, core_ids=list(range(NCORES)), trace=trace
    )
    out = np.concatenate(
        [
            np.ascontiguousarray(res.results[i]["out"].T.astype(np.float32))
            for i in range(NCORES)
        ],
        axis=0,
    )
    return out, res


def kernel(x, low, high, weight):
    x = np.asarray(x, np.float32)
    out, _ = run_cores(x, low, weight)
    out = host_correct_ties(out, x.reshape(-1), low, weight)
    return out.reshape(B, F, D)


# revision 12
# speedup vs baseline: 1.7590x; 1.7590x over previous
"""Trainium2 Bass kernel for nn_ContinuousEmbedding (histogram binning + distance-
weighted embedding mix).

Math: for each scalar x[b,f], the reference computes bucket index
idx = #{j in 1..63 : x > low[j]} and returns
    out[b,f,:] = sum_k weight[k,:] / (|idx-k|+1)  =  T[idx,:]
where T = S @ weight, S[i,k] = 1/(|i-k|+1) is a fixed 64x64 matrix.

T[idx] telescopes over compare signs s_j = sign(x - low[j]) (s_0 = +1 since
low[0] = -inf):
    T[idx] = sum_j s_j * V2[j],  V2[0] = (T[0]+T[63])/2, V2[j] = (T[j]-T[j-1])/2
i.e. out_row = V2^T s(x) -- a 64-deep fp16 contraction on the TensorEngine.

Per 2048-token chunk pair (32 pairs per core), tiles are [128, 1024] with chunk
A on partitions 0:64 and chunk B on 64:128:
  grid:  xb[128, 1024] = x broadcast to 64 partitions per chunk, via one of
           - gpsimd.partition_broadcast
           - DMA with a stride-0 (replicated) DRAM read
           - PE outer product from an exact 3-way fp16 split of x shipped from
             the host (hx+mx+lx == x exactly in f32 accumulation), K=3 matmul
         (mix per 8 pairs is tuned so GPSIMD / DMA / PE finish together)
  sign:  sg[128, 1024] fp16 = Sign(xb + (-low))  (ACT, per-partition bias; for
         the PE path the bias-add reads the PSUM x directly)
  gather: 4 matmuls vtab^T sg -> psum [128, 1024], diagonal PE quadrants
          (0,0)/(64,64) so both chunks' grids gather without moving data
  out:   DVE psum -> sbuf fp16 [128, 1024]; 2 HWDGE DMAs -> outT[64, NTOK]
The device output is transposed [D, tokens] fp16; the host transposes + casts
to f32 once at unshard time. Ties (x exactly equal to a bin edge) give
sign(0)=0 and are patched exactly on the host, as before.
"""

import os as _os
import sys

import numpy as np

for _p in ("/opt/trn_rl_repo",):
    if _p not in sys.path:
        sys.path.insert(0, _p)

import concourse.bass as bass  # noqa: E402,F401
import concourse.mybir as mybir  # noqa: E402
import concourse.tile as tile  # noqa: E402
from concourse import bacc  # noqa: E402
from concourse import bass_utils  # noqa: E402

B, F, K, D = 8192, 64, 64, 64
NCORES = 8
NTOK = (B // NCORES) * F          # 65536 tokens per core
CHUNK = 1024                      # tokens per chunk
NPAIR = NTOK // (2 * CHUNK)       # 32 chunk pairs
HALF = CHUNK // 2                 # tokens per matmul (N=512)

FP16 = mybir.dt.float16
F32 = mybir.dt.float32

CFG = {
    # mechanism per chunk pair, cycled: g = GPSIMD broadcast, d = DMA stride-0
    # broadcast, p = PE fp16-split outer product.
    "pat": "gdgpgdgp",
}
for _kv in _os.environ.get("KCFG", "").split(","):
    if "=" in _kv:
        _k, _v = _kv.split("=", 1)
        CFG[_k.strip()] = int(_v) if _v.strip().lstrip("-").isdigit() else _v.strip()

SIGN = mybir.ActivationFunctionType.Sign


def build_tile_kernel(nc, tc, x_d, xs_d, low_d, v_d, out_d):
    x_ap = x_d.ap().rearrange("(c n) -> c n", c=NTOK // CHUNK)       # [64, 1024]
    xs_ap = xs_d.ap().rearrange("k (p n) -> p k n", p=NPAIR)         # [32, 3, 2048]
    # fused pair store: [pair, (chunk d), n] <- ob[128, 1024]
    out_ap = out_d.ap().rearrange("d (c n) -> c d n", c=NTOK // CHUNK)

    pat = CFG["pat"]

    with tc.tile_pool(name="cpool", bufs=1) as cpool:
        neglow2 = cpool.tile([128, 1], F32)
        nc.sync.dma_start(out=neglow2[:], in_=low_d.ap())
        vtab2 = cpool.tile([128, D], FP16)
        nc.sync.dma_start(out=vtab2[:], in_=v_d.ap())
        ones3 = cpool.tile([3, K], FP16)
        nc.vector.memset(ones3[:], 1.0)

        with (
            tc.tile_pool(name="wpool", bufs=4) as wpool,
            tc.tile_pool(name="spool", bufs=4) as spool,
            tc.tile_pool(name="opool", bufs=4) as opool,
            tc.tile_pool(name="pxpool", bufs=2, space="PSUM") as pxpool,
            tc.tile_pool(name="popool", bufs=2, space="PSUM") as popool,
        ):
            for p in range(NPAIR):
                mech = pat[p % len(pat)]
                c0, c1 = 2 * p, 2 * p + 1

                if mech == "p":
                    # PE outer product from exact fp16 split; [128, 1024] packing
                    sg = spool.tile([128, CHUNK], FP16, tag="sg128")
                    xsp = wpool.tile([3, 2 * CHUNK], FP16, tag="xsp")
                    nc.sync.dma_start(out=xsp[:], in_=xs_ap[p])
                    px = pxpool.tile([128, CHUNK], F32, tag="px")
                    for h in range(2):
                        sl = slice(HALF * h, HALF * (h + 1))
                        nc.tensor.matmul(
                            out=px[0:64, sl], lhsT=ones3[:], rhs=xsp[:, sl],
                            start=True, stop=True, tile_position=(0, 0),
                        )
                        nc.tensor.matmul(
                            out=px[64:128, sl], lhsT=ones3[:],
                            rhs=xsp[:, CHUNK + HALF * h : CHUNK + HALF * (h + 1)],
                            start=True, stop=True, tile_position=(0, 64),
                        )
                    nc.scalar.activation(
                        out=sg[:], in_=px[:], func=SIGN, bias=neglow2[:], scale=1.0
                    )
                elif mech == "d":
                    # DMA stride-0 broadcast; [128, 1024] packing
                    sg = spool.tile([128, CHUNK], FP16, tag="sg128")
                    xb = wpool.tile([128, CHUNK], F32, tag="xb128")
                    nc.sync.dma_start(
                        out=xb[0:64, :],
                        in_=x_ap[c0].unsqueeze(0).broadcast_to([K, CHUNK]),
                    )
                    nc.sync.dma_start(
                        out=xb[64:128, :],
                        in_=x_ap[c1].unsqueeze(0).broadcast_to([K, CHUNK]),
                    )
                    nc.scalar.activation(
                        out=sg[:], in_=xb[:], func=SIGN, bias=neglow2[:], scale=1.0
                    )
                else:
                    # GPSIMD broadcast: must stay on partitions 0:64 -> [64, 2048]
                    sg = spool.tile([K, 2 * CHUNK], FP16, tag="sg64")
                    xb = wpool.tile([K, 2 * CHUNK], F32, tag="xb64")
                    xr0 = wpool.tile([1, CHUNK], F32, tag="xr0")
                    xr1 = wpool.tile([1, CHUNK], F32, tag="xr1")
                    nc.sync.dma_start(out=xr0[:], in_=x_ap[c0])
                    nc.sync.dma_start(out=xr1[:], in_=x_ap[c1])
                    nc.gpsimd.partition_broadcast(xb[:, 0:CHUNK], xr0[:], channels=K)
                    nc.gpsimd.partition_broadcast(
                        xb[:, CHUNK : 2 * CHUNK], xr1[:], channels=K
                    )
                    nc.scalar.activation(
                        out=sg[:], in_=xb[:], func=SIGN,
                        bias=neglow2[0:64, :], scale=1.0,
                    )

                po = popool.tile([128, CHUNK], F32, tag="po")
                for h in range(2):
                    sl = slice(HALF * h, HALF * (h + 1))
                    slB = slice(CHUNK + HALF * h, CHUNK + HALF * (h + 1))
                    if mech == "g":
                        nc.tensor.matmul(
                            out=po[0:64, sl], lhsT=vtab2[0:64, :], rhs=sg[:, sl],
                            start=True, stop=True, tile_position=(0, 0),
                        )
                        nc.tensor.matmul(
                            out=po[64:128, sl], lhsT=vtab2[0:64, :], rhs=sg[:, slB],
                            start=True, stop=True, tile_position=(0, 64),
                        )
                    else:
                        nc.tensor.matmul(
                            out=po[0:64, sl], lhsT=vtab2[0:64, :], rhs=sg[0:64, sl],
                            start=True, stop=True, tile_position=(0, 0),
                        )
                        nc.tensor.matmul(
                            out=po[64:128, sl], lhsT=vtab2[64:128, :],
                            rhs=sg[64:128, sl],
                            start=True, stop=True, tile_position=(64, 64),
                        )

                ob = opool.tile([128, CHUNK], FP16, tag="ob")
                nc.vector.tensor_copy(out=ob[:], in_=po[:])
                nc.sync.dma_start(out=out_ap[c0], in_=ob[0:64, :])
                nc.sync.dma_start(out=out_ap[c1], in_=ob[64:128, :])


_CACHED_NC = None


def _get_nc():
    global _CACHED_NC
    if _CACHED_NC is None:
        nc = bacc.Bacc("TRN2", target_bir_lowering=False, debug=False)
        x_d = nc.dram_tensor("x", [NTOK], F32, kind="ExternalInput")
        xs_d = nc.dram_tensor("xsplit", [3, NTOK], FP16, kind="ExternalInput")
        low_d = nc.dram_tensor("lowcol", [128, 1], F32, kind="ExternalInput")
        v_d = nc.dram_tensor("vtab", [128, D], FP16, kind="ExternalInput")
        out_d = nc.dram_tensor("out", [D, NTOK], FP16, kind="ExternalOutput")
        with tile.TileContext(nc) as tc:
            build_tile_kernel(nc, tc, x_d, xs_d, low_d, v_d, out_d)
        nc.compile()
        _CACHED_NC = nc
    return _CACHED_NC


def make_host_tables(low, weight):
    """V2 duplicated to [128, D] fp16 (sign-telescoped table) and -low column
    duplicated to [128, 1] f32, computed in float64."""
    ar = np.arange(K)
    S = 1.0 / (np.abs(ar[:, None] - ar[None, :]) + 1.0)              # [K, K] f64
    T = S @ np.asarray(weight, np.float64)                           # [K, D]
    V = np.empty_like(T)
    V[0] = (T[0] + T[-1]) / 2
    V[1:] = (T[1:] - T[:-1]) / 2
    vtab = V.astype(np.float16)
    vtab2 = np.concatenate([vtab, vtab], axis=0)                     # [128, D]
    lowcol = (-np.asarray(low, np.float64)).astype(np.float32).reshape(K, 1)
    lowcol2 = np.concatenate([lowcol, lowcol], axis=0)               # [128, 1]
    return lowcol2, vtab2


def split_fp16_3(xflat):
    """Exact 3-way fp16 split: hx + mx + lx == x in f32 (24 <= 3*11 mantissa
    bits; each partial sum is exactly representable)."""
    x = np.asarray(xflat, np.float32)
    hx = x.astype(np.float16)
    r = x - hx.astype(np.float32)
    mx = r.astype(np.float16)
    r2 = r - mx.astype(np.float32)
    lx = r2.astype(np.float16)
    return np.stack([hx, mx, lx], axis=0)                            # [3, NTOK]


def host_correct_ties(out2d, xflat, low, weight):
    """Exact fixup for tokens where x equals a bin edge: the device Sign gives
    sign(0)=0 there (averaging two table rows) while the reference uses strict
    x > low. Replace those few rows with the exact table row."""
    bins = np.asarray(low, np.float32)[1:]
    ties = np.isin(xflat, bins)
    if not ties.any():
        return out2d
    xt = xflat[ties]
    idx = (xt[:, None] > bins[None, :]).sum(-1)
    ar = np.arange(K)
    S = 1.0 / (np.abs(ar[:, None] - ar[None, :]) + 1.0)
    T = (S @ np.asarray(weight, np.float64)).astype(np.float32)
    out2d[ties] = T[idx]
    return out2d


def make_in_maps(x, low, weight):
    lowcol2, vtab2 = make_host_tables(low, weight)
    shards = np.asarray(x, np.float32).reshape(NCORES, NTOK)
    in_maps = []
    for i in range(NCORES):
        xi = np.ascontiguousarray(shards[i])
        in_maps.append(
            {"x": xi, "xsplit": split_fp16_3(xi), "lowcol": lowcol2, "vtab": vtab2}
        )
    return in_maps


def run_cores(x, low, weight, trace=False):
    """Shard, run on 8 cores, return ([NTOK*8, D] f32 output, BassKernelResults)."""
    nc = _get_nc()
    in_maps = make_in_maps(x, low, weight)
    res = bass_utils.run_bass_kernel_spmd(
        nc, in_maps, core_ids=list(range(NCORES)), trace=trace
    )
    out = np.concatenate(
        [
            np.ascontiguousarray(res.results[i]["out"].T.astype(np.float32))
            for i in range(NCORES)
        ],
        axis=0,
    )
    return out, res


def kernel(x, low, high, weight):
    x = np.asarray(x, np.float32)
    out, _ = run_cores(x, low, weight)
    out = host_correct_ties(out, x.reshape(-1), low, weight)
    return out.reshape(B, F, D)


# revision 19
# speedup vs baseline: 1.9292x; 1.0967x over previous
"""Trainium2 Bass kernel for nn_ContinuousEmbedding (histogram binning + distance-
weighted embedding mix).

Math: for each scalar x[b,f], the reference computes bucket index
idx = #{j in 1..63 : x > low[j]} and returns
    out[b,f,:] = sum_k weight[k,:] / (|idx-k|+1)  =  T[idx,:]
where T = S @ weight, S[i,k] = 1/(|i-k|+1) is a fixed 64x64 matrix.

T[idx] telescopes over compare signs s_j = sign(x - low[j]) (s_0 = +1 since
low[0] = -inf):
    T[idx] = sum_j s_j * V2[j],  V2[0] = (T[0]+T[63])/2, V2[j] = (T[j]-T[j-1])/2
i.e. out_row = V2^T s(x) -- a 64-deep fp16 contraction on the TensorEngine.

Per 2048-token chunk pair (32 pairs per core), tiles are [128, 1024] with chunk
A on partitions 0:64 and chunk B on 64:128:
  grid:  xb[128, 1024] = x broadcast to 64 partitions per chunk, via one of
           - gpsimd.partition_broadcast
           - DMA with a stride-0 (replicated) DRAM read
           - PE outer product from an exact 3-way fp16 split of x shipped from
             the host (hx+mx+lx == x exactly in f32 accumulation), K=3 matmul
         (mix per 8 pairs is tuned so GPSIMD / DMA / PE finish together)
  sign:  sg[128, 1024] fp16 = Sign(xb + (-low))  (ACT, per-partition bias; for
         the PE path the bias-add reads the PSUM x directly)
  gather: 4 matmuls vtab^T sg -> psum [128, 1024], diagonal PE quadrants
          (0,0)/(64,64) so both chunks' grids gather without moving data
  out:   DVE psum -> sbuf fp16 [128, 1024]; 2 HWDGE DMAs -> outT[64, NTOK]
The device output is transposed [D, tokens] fp16; the host transposes + casts
to f32 once at unshard time. Ties (x exactly equal to a bin edge) give
sign(0)=0 and are patched exactly on the host, as before.
"""

import os as _os
import sys

import numpy as np

for _p in ("/opt/trn_rl_repo",):
    if _p not in sys.path:
        sys.path.insert(0, _p)

import concourse.bass as bass  # noqa: E402,F401
import concourse.mybir as mybir  # noqa: E402
import concourse.tile as tile  # noqa: E402
from concourse import bacc  # noqa: E402
from concourse import bass_utils  # noqa: E402

B, F, K, D = 8192, 64, 64, 64
NCORES = 8
NTOK = (B // NCORES) * F          # 65536 tokens per core
CHUNK = 1024                      # tokens per chunk
NPAIR = NTOK // (2 * CHUNK)       # 32 chunk pairs
HALF = CHUNK // 2                 # tokens per matmul (N=512)

FP16 = mybir.dt.float16
F32 = mybir.dt.float32

CFG = {
    # mechanism per chunk pair, cycled: g = GPSIMD broadcast, d = DMA stride-0
    # broadcast, p = PE fp16-split outer product.
    "pat": "gdgpgdgp",
}
for _kv in _os.environ.get("KCFG", "").split(","):
    if "=" in _kv:
        _k, _v = _kv.split("=", 1)
        CFG[_k.strip()] = int(_v) if _v.strip().lstrip("-").isdigit() else _v.strip()

SIGN = mybir.ActivationFunctionType.Sign


def build_tile_kernel(nc, tc, x_d, xs_d, low_d, v_d, out_d):
    x_ap = x_d.ap().rearrange("(c n) -> c n", c=NTOK // CHUNK)       # [64, 1024]
    xg_ap = x_d.ap().rearrange("(g n) -> g n", g=NPAIR // 4)         # [8, 8192]
    xs_ap = xs_d.ap().rearrange("k (p n) -> p k n", p=NPAIR)         # [32, 3, 2048]
    # device out layout [128, NTOK/2]: pair p cols p*1024..+1024, row r<64 =
    # chunk 2p dim r, row r>=64 = chunk 2p+1 dim r-64; host untangles.
    out_ap = out_d.ap().rearrange("r (p n) -> p r n", n=CHUNK)       # [32, 128, 1024]

    pat = CFG["pat"]

    with tc.tile_pool(name="cpool", bufs=1) as cpool:
        neglow2 = cpool.tile([128, 1], F32)
        nc.sync.dma_start(out=neglow2[:], in_=low_d.ap())
        vtab2 = cpool.tile([128, D], FP16)
        nc.sync.dma_start(out=vtab2[:], in_=v_d.ap())
        ones3 = cpool.tile([3, K], FP16)
        nc.vector.memset(ones3[:], 1.0)

        with (
            tc.tile_pool(name="wpool", bufs=4) as wpool,
            tc.tile_pool(name="spool", bufs=4) as spool,
            tc.tile_pool(name="opool", bufs=4) as opool,
            tc.tile_pool(name="pxpool", bufs=2, space="PSUM") as pxpool,
            tc.tile_pool(name="popool", bufs=2, space="PSUM") as popool,
        ):
            xr8 = None
            for p in range(NPAIR):
                mech = pat[p % len(pat)]
                c0, c1 = 2 * p, 2 * p + 1
                if p % 4 == 0 and "g" in pat:
                    xr8 = wpool.tile([1, 8192], F32, tag="xr8", bufs=2)
                    nc.sync.dma_start(out=xr8[:], in_=xg_ap[p // 4])
                po2 = (p % 4) * 2 * CHUNK                            # offset in xr8

                if mech == "p":
                    # PE outer product from exact fp16 split; [128, 1024] packing
                    sg = spool.tile([128, CHUNK], FP16, tag="sg128")
                    xsp = wpool.tile([3, 2 * CHUNK], FP16, tag="xsp")
                    nc.sync.dma_start(out=xsp[:], in_=xs_ap[p])
                    px = pxpool.tile([128, CHUNK], F32, tag="px")
                    for h in range(2):
                        sl = slice(HALF * h, HALF * (h + 1))
                        nc.tensor.matmul(
                            out=px[0:64, sl], lhsT=ones3[:], rhs=xsp[:, sl],
                            start=True, stop=True, tile_position=(0, 0),
                        )
                        nc.tensor.matmul(
                            out=px[64:128, sl], lhsT=ones3[:],
                            rhs=xsp[:, CHUNK + HALF * h : CHUNK + HALF * (h + 1)],
                            start=True, stop=True, tile_position=(0, 64),
                        )
                    nc.scalar.activation(
                        out=sg[:], in_=px[:], func=SIGN, bias=neglow2[:], scale=1.0
                    )
                elif mech == "d":
                    # DMA stride-0 broadcast; [128, 1024] packing; one 3-D DMA
                    sg = spool.tile([128, CHUNK], FP16, tag="sg128")
                    xb = wpool.tile([128, CHUNK], F32, tag="xb128")
                    nc.sync.dma_start(
                        out=xb[0:64, :],
                        in_=x_ap[c0].unsqueeze(0).broadcast_to([K, CHUNK]),
                    )
                    nc.sync.dma_start(
                        out=xb[64:128, :],
                        in_=x_ap[c1].unsqueeze(0).broadcast_to([K, CHUNK]),
                    )
                    nc.scalar.activation(
                        out=sg[:], in_=xb[:], func=SIGN, bias=neglow2[:], scale=1.0
                    )
                else:
                    # GPSIMD broadcast: must stay on partitions 0:64 -> [64, 2048]
                    sg = spool.tile([K, 2 * CHUNK], FP16, tag="sg64")
                    xb = wpool.tile([K, 2 * CHUNK], F32, tag="xb64")
                    nc.gpsimd.partition_broadcast(
                        xb[:, 0:CHUNK], xr8[:, po2 : po2 + CHUNK], channels=K
                    )
                    nc.gpsimd.partition_broadcast(
                        xb[:, CHUNK : 2 * CHUNK],
                        xr8[:, po2 + CHUNK : po2 + 2 * CHUNK],
                        channels=K,
                    )
                    nc.scalar.activation(
                        out=sg[:], in_=xb[:], func=SIGN,
                        bias=neglow2[0:64, :], scale=1.0,
                    )

                po = popool.tile([128, CHUNK], F32, tag="po")
                for h in range(2):
                    sl = slice(HALF * h, HALF * (h + 1))
                    slB = slice(CHUNK + HALF * h, CHUNK + HALF * (h + 1))
                    if mech == "g":
                        nc.tensor.matmul(
                            out=po[0:64, sl], lhsT=vtab2[0:64, :], rhs=sg[:, sl],
                            start=True, stop=True, tile_position=(0, 0),
                        )
                        nc.tensor.matmul(
                            out=po[64:128, sl], lhsT=vtab2[0:64, :], rhs=sg[:, slB],
                            start=True, stop=True, tile_position=(0, 64),
                        )
                    else:
                        nc.tensor.matmul(
                            out=po[0:64, sl], lhsT=vtab2[0:64, :], rhs=sg[0:64, sl],
                            start=True, stop=True, tile_position=(0, 0),
                        )
                        nc.tensor.matmul(
                            out=po[64:128, sl], lhsT=vtab2[64:128, :],
                            rhs=sg[64:128, sl],
                            start=True, stop=True, tile_position=(64, 64),
                        )

                ob = opool.tile([128, CHUNK], FP16, tag="ob")
                nc.vector.tensor_copy(out=ob[:], in_=po[:])
                nc.sync.dma_start(out=out_ap[p], in_=ob[:])


_CACHED_NC = None


def _get_nc():
    global _CACHED_NC
    if _CACHED_NC is None:
        nc = bacc.Bacc("TRN2", target_bir_lowering=False, debug=False)
        x_d = nc.dram_tensor("x", [NTOK], F32, kind="ExternalInput")
        xs_d = nc.dram_tensor("xsplit", [3, NTOK], FP16, kind="ExternalInput")
        low_d = nc.dram_tensor("lowcol", [128, 1], F32, kind="ExternalInput")
        v_d = nc.dram_tensor("vtab", [128, D], FP16, kind="ExternalInput")
        out_d = nc.dram_tensor("out", [128, NTOK // 2], FP16, kind="ExternalOutput")
        with tile.TileContext(nc) as tc:
            build_tile_kernel(nc, tc, x_d, xs_d, low_d, v_d, out_d)
        nc.compile()
        _CACHED_NC = nc
    return _CACHED_NC


def make_host_tables(low, weight):
    """V2 duplicated to [128, D] fp16 (sign-telescoped table) and -low column
    duplicated to [128, 1] f32, computed in float64."""
    ar = np.arange(K)
    S = 1.0 / (np.abs(ar[:, None] - ar[None, :]) + 1.0)              # [K, K] f64
    T = S @ np.asarray(weight, np.float64)                           # [K, D]
    V = np.empty_like(T)
    V[0] = (T[0] + T[-1]) / 2
    V[1:] = (T[1:] - T[:-1]) / 2
    vtab = V.astype(np.float16)
    vtab2 = np.concatenate([vtab, vtab], axis=0)                     # [128, D]
    lowcol = (-np.asarray(low, np.float64)).astype(np.float32).reshape(K, 1)
    lowcol2 = np.concatenate([lowcol, lowcol], axis=0)               # [128, 1]
    return lowcol2, vtab2


def split_fp16_3(xflat):
    """Exact 3-way fp16 split: hx + mx + lx == x in f32 (24 <= 3*11 mantissa
    bits; each partial sum is exactly representable)."""
    x = np.asarray(xflat, np.float32)
    hx = x.astype(np.float16)
    r = x - hx.astype(np.float32)
    mx = r.astype(np.float16)
    r2 = r - mx.astype(np.float32)
    lx = r2.astype(np.float16)
    return np.stack([hx, mx, lx], axis=0)                            # [3, NTOK]


def host_correct_ties(out2d, xflat, low, weight):
    """Exact fixup for tokens where x equals a bin edge: the device Sign gives
    sign(0)=0 there (averaging two table rows) while the reference uses strict
    x > low. Replace those few rows with the exact table row."""
    bins = np.asarray(low, np.float32)[1:]
    ties = np.isin(xflat, bins)
    if not ties.any():
        return out2d
    xt = xflat[ties]
    idx = (xt[:, None] > bins[None, :]).sum(-1)
    ar = np.arange(K)
    S = 1.0 / (np.abs(ar[:, None] - ar[None, :]) + 1.0)
    T = (S @ np.asarray(weight, np.float64)).astype(np.float32)
    out2d[ties] = T[idx]
    return out2d


def make_in_maps(x, low, weight):
    lowcol2, vtab2 = make_host_tables(low, weight)
    shards = np.asarray(x, np.float32).reshape(NCORES, NTOK)
    in_maps = []
    for i in range(NCORES):
        xi = np.ascontiguousarray(shards[i])
        in_maps.append(
            {"x": xi, "xsplit": split_fp16_3(xi), "lowcol": lowcol2, "vtab": vtab2}
        )
    return in_maps


def run_cores(x, low, weight, trace=False):
    """Shard, run on 8 cores, return ([NTOK*8, D] f32 output, BassKernelResults)."""
    nc = _get_nc()
    in_maps = make_in_maps(x, low, weight)
    res = bass_utils.run_bass_kernel_spmd(
        nc, in_maps, core_ids=list(range(NCORES)), trace=trace
    )
    outs = []
    for i in range(NCORES):
        o = res.results[i]["out"]                       # [128, NTOK//2] fp16
        arr = o.reshape(2, D, NPAIR, CHUNK)             # [two, d, pair, n]
        outs.append(
            np.ascontiguousarray(
                np.transpose(arr, (2, 0, 3, 1)).reshape(NTOK, D)
            ).astype(np.float32)
        )
    return np.concatenate(outs, axis=0), res


def kernel(x, low, high, weight):
    x = np.asarray(x, np.float32)
    out, _ = run_cores(x, low, weight)
    out = host_correct_ties(out, x.reshape(-1), low, weight)
    return out.reshape(B, F, D)


# revision 24
# speedup vs baseline: 2.3384x; 1.2121x over previous
"""Trainium2 Bass kernel for nn_ContinuousEmbedding (histogram binning + distance-
weighted embedding mix).

Math: for each scalar x[b,f], the reference computes bucket index
idx = #{j in 1..63 : x > low[j]} and returns
    out[b,f,:] = sum_k weight[k,:] / (|idx-k|+1)  =  T[idx,:]
where T = S @ weight, S[i,k] = 1/(|i-k|+1) is a fixed 64x64 matrix.

T[idx] telescopes over compare signs s_j = sign(x - low[j]) (s_0 = +1 since
low[0] = -inf):
    T[idx] = sum_j s_j * V2[j],  V2[0] = (T[0]+T[63])/2, V2[j] = (T[j]-T[j-1])/2
i.e. out_row = V2^T s(x) -- a 64-deep fp16 contraction on the TensorEngine.

Per 2048-token chunk pair (32 pairs per core), tiles are [128, 1024] with chunk
A on partitions 0:64 and chunk B on 64:128:
  grid:  xb[128, 1024] = x broadcast to 64 partitions per chunk, via one of
           - gpsimd.partition_broadcast
           - DMA with a stride-0 (replicated) DRAM read
           - PE outer product from an exact 3-way fp16 split of x shipped from
             the host (hx+mx+lx == x exactly in f32 accumulation), K=3 matmul
         (mix per 8 pairs is tuned so GPSIMD / DMA / PE finish together)
  sign:  sg[128, 1024] fp16 = Sign(xb + (-low))  (ACT, per-partition bias; for
         the PE path the bias-add reads the PSUM x directly)
  gather: 4 matmuls vtab^T sg -> psum [128, 1024], diagonal PE quadrants
          (0,0)/(64,64) so both chunks' grids gather without moving data
  out:   DVE psum -> sbuf fp16 [128, 1024]; 2 HWDGE DMAs -> outT[64, NTOK]
The device output is transposed [D, tokens] fp16; the host transposes + casts
to f32 once at unshard time. Ties (x exactly equal to a bin edge) give
sign(0)=0 and are patched exactly on the host, as before.
"""

import os as _os
import sys

import numpy as np

for _p in ("/opt/trn_rl_repo",):
    if _p not in sys.path:
        sys.path.insert(0, _p)

import concourse.bass as bass  # noqa: E402,F401
import concourse.mybir as mybir  # noqa: E402
import concourse.tile as tile  # noqa: E402
from concourse import bacc  # noqa: E402
from concourse import bass_utils  # noqa: E402

B, F, K, D = 8192, 64, 64, 64
NCORES = 8
NTOK = (B // NCORES) * F          # 65536 tokens per core
CHUNK = 1024                      # tokens per chunk
NPAIR = NTOK // (2 * CHUNK)       # 32 chunk pairs
HALF = CHUNK // 2                 # tokens per matmul (N=512)

FP16 = mybir.dt.float16
F32 = mybir.dt.float32

CFG = {
    # mechanism per chunk pair, cycled: g = GPSIMD broadcast, d = DMA stride-0
    # broadcast, p = PE fp16-split outer product.
    "pat": "gdgpgdgp",
}
for _kv in _os.environ.get("KCFG", "").split(","):
    if "=" in _kv:
        _k, _v = _kv.split("=", 1)
        CFG[_k.strip()] = int(_v) if _v.strip().lstrip("-").isdigit() else _v.strip()

SIGN = mybir.ActivationFunctionType.Sign


def build_tile_kernel(nc, tc, x_d, xs_d, low_d, v_d, out_d):
    x_ap = x_d.ap().rearrange("(c n) -> c n", c=NTOK // CHUNK)       # [64, 1024]
    xg_ap = x_d.ap().rearrange("(g n) -> g n", g=NPAIR // 4)         # [8, 8192]
    xs_ap = xs_d.ap().rearrange("k (p n) -> p k n", p=NPAIR)         # [32, 3, 2048]
    # device out layout [128, NTOK/2]: pair p cols p*1024..+1024, row r<64 =
    # chunk 2p dim r, row r>=64 = chunk 2p+1 dim r-64; host untangles.
    out2_ap = out_d.ap().rearrange("r (p n) -> p r n", n=2 * CHUNK)  # [16, 128, 2048]

    pat = CFG["pat"]

    with tc.tile_pool(name="cpool", bufs=1) as cpool:
        neglow2 = cpool.tile([128, 1], F32)
        nc.sync.dma_start(out=neglow2[:], in_=low_d.ap())
        vtab2 = cpool.tile([128, D], FP16)
        nc.sync.dma_start(out=vtab2[:], in_=v_d.ap())
        ones3 = cpool.tile([3, K], FP16)
        nc.vector.memset(ones3[:], 1.0)

        with (
            tc.tile_pool(name="wpool", bufs=4) as wpool,
            tc.tile_pool(name="spool", bufs=4) as spool,
            tc.tile_pool(name="opool", bufs=4) as opool,
            tc.tile_pool(name="pxpool", bufs=2, space="PSUM") as pxpool,
            tc.tile_pool(name="popool", bufs=2, space="PSUM") as popool,
        ):
            mechs = [pat[p % len(pat)] for p in range(NPAIR)]
            LOOK = 2                   # input prefetch distance (pairs)
            pend_xb, pend_xsp, xr8s = {}, {}, {}

            def prefetch(q):
                if q >= NPAIR:
                    return
                if q % 4 == 0 and "g" in pat:
                    t = wpool.tile([1, 8192], F32, tag="xr8", bufs=3)
                    nc.sync.dma_start(out=t[:], in_=xg_ap[q // 4])
                    xr8s[q // 4] = t
                if mechs[q] == "d":
                    t = wpool.tile([128, CHUNK], F32, tag="xb128")
                    nc.sync.dma_start(
                        out=t[0:64, :],
                        in_=x_ap[2 * q].unsqueeze(0).broadcast_to([K, CHUNK]),
                    )
                    nc.sync.dma_start(
                        out=t[64:128, :],
                        in_=x_ap[2 * q + 1].unsqueeze(0).broadcast_to([K, CHUNK]),
                    )
                    pend_xb[q] = t
                elif mechs[q] == "p":
                    t = wpool.tile([3, 2 * CHUNK], FP16, tag="xsp")
                    nc.sync.dma_start(out=t[:], in_=xs_ap[q])
                    pend_xsp[q] = t

            for q in range(LOOK):
                prefetch(q)
            ob2 = None
            for p in range(NPAIR):
                mech = mechs[p]
                c0, c1 = 2 * p, 2 * p + 1
                prefetch(p + LOOK)
                xr8 = xr8s.get(p // 4)
                po2 = (p % 4) * 2 * CHUNK                            # offset in xr8

                if mech == "p":
                    # PE outer product from exact fp16 split; [128, 1024] packing
                    sg = spool.tile([128, CHUNK], FP16, tag="sg128")
                    xsp = pend_xsp.pop(p)
                    px = pxpool.tile([128, CHUNK], F32, tag="px")
                    for h in range(2):
                        sl = slice(HALF * h, HALF * (h + 1))
                        nc.tensor.matmul(
                            out=px[0:64, sl], lhsT=ones3[:], rhs=xsp[:, sl],
                            start=True, stop=True, tile_position=(0, 0),
                        )
                        nc.tensor.matmul(
                            out=px[64:128, sl], lhsT=ones3[:],
                            rhs=xsp[:, CHUNK + HALF * h : CHUNK + HALF * (h + 1)],
                            start=True, stop=True, tile_position=(0, 64),
                        )
                    nc.scalar.activation(
                        out=sg[:], in_=px[:], func=SIGN, bias=neglow2[:], scale=1.0
                    )
                elif mech == "d":
                    # DMA stride-0 broadcast; [128, 1024] packing (prefetched)
                    sg = spool.tile([128, CHUNK], FP16, tag="sg128")
                    xb = pend_xb.pop(p)
                    nc.scalar.activation(
                        out=sg[:], in_=xb[:], func=SIGN, bias=neglow2[:], scale=1.0
                    )
                else:
                    # GPSIMD broadcast: must stay on partitions 0:64 -> [64, 2048]
                    sg = spool.tile([K, 2 * CHUNK], FP16, tag="sg64")
                    xb = wpool.tile([K, 2 * CHUNK], F32, tag="xb64")
                    nc.gpsimd.partition_broadcast(
                        xb[:, 0:CHUNK], xr8[:, po2 : po2 + CHUNK], channels=K
                    )
                    nc.gpsimd.partition_broadcast(
                        xb[:, CHUNK : 2 * CHUNK],
                        xr8[:, po2 + CHUNK : po2 + 2 * CHUNK],
                        channels=K,
                    )
                    nc.scalar.activation(
                        out=sg[:], in_=xb[:], func=SIGN,
                        bias=neglow2[0:64, :], scale=1.0,
                    )

                po = popool.tile([128, CHUNK], F32, tag="po")
                for h in range(2):
                    sl = slice(HALF * h, HALF * (h + 1))
                    slB = slice(CHUNK + HALF * h, CHUNK + HALF * (h + 1))
                    if mech == "g":
                        nc.tensor.matmul(
                            out=po[0:64, sl], lhsT=vtab2[0:64, :], rhs=sg[:, sl],
                            start=True, stop=True, tile_position=(0, 0),
                        )
                        nc.tensor.matmul(
                            out=po[64:128, sl], lhsT=vtab2[0:64, :], rhs=sg[:, slB],
                            start=True, stop=True, tile_position=(0, 64),
                        )
                    else:
                        nc.tensor.matmul(
                            out=po[0:64, sl], lhsT=vtab2[0:64, :], rhs=sg[0:64, sl],
                            start=True, stop=True, tile_position=(0, 0),
                        )
                        nc.tensor.matmul(
                            out=po[64:128, sl], lhsT=vtab2[64:128, :],
                            rhs=sg[64:128, sl],
                            start=True, stop=True, tile_position=(64, 64),
                        )

                if p % 2 == 0:
                    ob2 = opool.tile([128, 2 * CHUNK], FP16, tag="ob2")
                nc.vector.tensor_copy(
                    out=ob2[:, (p % 2) * CHUNK : (p % 2 + 1) * CHUNK], in_=po[:]
                )
                if p % 2 == 1:
                    nc.sync.dma_start(out=out2_ap[p // 2], in_=ob2[:])


_CACHED_NC = None


def _get_nc():
    global _CACHED_NC
    if _CACHED_NC is None:
        nc = bacc.Bacc("TRN2", target_bir_lowering=False, debug=False)
        x_d = nc.dram_tensor("x", [NTOK], F32, kind="ExternalInput")
        xs_d = nc.dram_tensor("xsplit", [3, NTOK], FP16, kind="ExternalInput")
        low_d = nc.dram_tensor("lowcol", [128, 1], F32, kind="ExternalInput")
        v_d = nc.dram_tensor("vtab", [128, D], FP16, kind="ExternalInput")
        out_d = nc.dram_tensor("out", [128, NTOK // 2], FP16, kind="ExternalOutput")
        with tile.TileContext(nc) as tc:
            build_tile_kernel(nc, tc, x_d, xs_d, low_d, v_d, out_d)
        nc.compile()
        _CACHED_NC = nc
    return _CACHED_NC


def make_host_tables(low, weight):
    """V2 duplicated to [128, D] fp16 (sign-telescoped table) and -low column
    duplicated to [128, 1] f32, computed in float64."""
    ar = np.arange(K)
    S = 1.0 / (np.abs(ar[:, None] - ar[None, :]) + 1.0)              # [K, K] f64
    T = S @ np.asarray(weight, np.float64)                           # [K, D]
    V = np.empty_like(T)
    V[0] = (T[0] + T[-1]) / 2
    V[1:] = (T[1:] - T[:-1]) / 2
    vtab = V.astype(np.float16)
    vtab2 = np.concatenate([vtab, vtab], axis=0)                     # [128, D]
    lowcol = (-np.asarray(low, np.float64)).astype(np.float32).reshape(K, 1)
    lowcol2 = np.concatenate([lowcol, lowcol], axis=0)               # [128, 1]
    return lowcol2, vtab2


def split_fp16_3(xflat):
    """Exact 3-way fp16 split: hx + mx + lx == x in f32 (24 <= 3*11 mantissa
    bits; each partial sum is exactly representable)."""
    x = np.asarray(xflat, np.float32)
    hx = x.astype(np.float16)
    r = x - hx.astype(np.float32)
    mx = r.astype(np.float16)
    r2 = r - mx.astype(np.float32)
    lx = r2.astype(np.float16)
    return np.stack([hx, mx, lx], axis=0)                            # [3, NTOK]


def host_correct_ties(out2d, xflat, low, weight):
    """Exact fixup for tokens where x equals a bin edge: the device Sign gives
    sign(0)=0 there (averaging two table rows) while the reference uses strict
    x > low. Replace those few rows with the exact table row."""
    bins = np.asarray(low, np.float32)[1:]
    ties = np.isin(xflat, bins)
    if not ties.any():
        return out2d
    xt = xflat[ties]
    idx = (xt[:, None] > bins[None, :]).sum(-1)
    ar = np.arange(K)
    S = 1.0 / (np.abs(ar[:, None] - ar[None, :]) + 1.0)
    T = (S @ np.asarray(weight, np.float64)).astype(np.float32)
    out2d[ties] = T[idx]
    return out2d


def make_in_maps(x, low, weight):
    lowcol2, vtab2 = make_host_tables(low, weight)
    shards = np.asarray(x, np.float32).reshape(NCORES, NTOK)
    in_maps = []
    for i in range(NCORES):
        xi = np.ascontiguousarray(shards[i])
        in_maps.append(
            {"x": xi, "xsplit": split_fp16_3(xi), "lowcol": lowcol2, "vtab": vtab2}
        )
    return in_maps


def run_cores(x, low, weight, trace=False):
    """Shard, run on 8 cores, return ([NTOK*8, D] f32 output, BassKernelResults)."""
    nc = _get_nc()
    in_maps = make_in_maps(x, low, weight)
    res = bass_utils.run_bass_kernel_spmd(
        nc, in_maps, core_ids=list(range(NCORES)), trace=trace
    )
    outs = []
    for i in range(NCORES):
        o = res.results[i]["out"]                       # [128, NTOK//2] fp16
        arr = o.reshape(2, D, NPAIR, CHUNK)             # [two, d, pair, n]
        outs.append(
            np.ascontiguousarray(
                np.transpose(arr, (2, 0, 3, 1)).reshape(NTOK, D)
            ).astype(np.float32)
        )
    return np.concatenate(outs, axis=0), res


def kernel(x, low, high, weight):
    x = np.asarray(x, np.float32)
    out, _ = run_cores(x, low, weight)
    out = host_correct_ties(out, x.reshape(-1), low, weight)
    return out.reshape(B, F, D)
